# revision 12
# baseline (speedup 1.0000x reference)
"""ChunkRanker Bass kernel for Trainium2, 8-core data-parallel.

Math per chunk n (chunks: [4096, 128, 64] f32):
  flat = chunks[n].reshape(8192)
  std  = std(flat, ddof=1)
  realism = std<0.01 ? 10*std : (std>0.5 ? 0.5/std : 1-|std-0.1|)
  ctx    = previous_context[-10:].flatten()            # [640]
  starts = flat[:640]
  boundary = dot(starts, ctx) / max(|starts|*|ctx|, 1e-8)
  score = realism + 0.3*0.5 + 0.2*boundary

Sharding: leading chunk axis split 8 ways (512 chunks/core); ctx broadcast.

Per-core layout: 4 static SBUF tiles of [128 partitions = chunks, 8192 free =
chunk elements] (16 MB -- the whole local input stays resident, so every DMA
is enqueued up front and the 16 queues stream at full rate, ~441 GB/s, with
no buffer-reuse stalls). Tiles stream tile-major in ~1 MB pieces.

Reduction split per tile (std needs sum AND sumsq = 2 moments per chunk):
  - DVE bn_stats yields BOTH moments in one pass (593ns/512-group): boundary
    [0,640) as 2x320 groups (whose stats also give |starts|^2 for free) plus
    8x512 groups over [4096,8192); dot(starts,ctx) via STT accum.
  - ACT covers [640,4096) with two accumulating passes (Square then Copy) at
    1.2GHz -- both moments, one engine.
Each engine runs ~7-9us per ~10.5us tile window, so the kernel is DMA-bound.

Pipeline discipline: every tile's stats conversion + scalar chain + output
column DMA runs in the NEXT tile's stream window (per-tile chains, so no
multi-tile dependency pushes work to the end). Tile 3 splits its ACT region
in two sub-regions that fire per piece (both drain before the stream ends),
pre-folds bn groups 0-8 + ACT accums into partials mid-stream, and ends on a
single 512 bn-group: after the final byte lands only [1 bn_stats + tiny
combines + a [128,1] scalar chain + 2KB out-DMA] remain. The chain's three
affine branch values are computed on ACT (Copy with scale/bias) to keep DVE
ops off the critical path.

bn group stats [c,m,c*var] convert to (sum, sumsq): sum_g = c*m,
ssq_g = c*var + (c*m)*m.

A dummy sqrt at kernel start pins the sqrt_and_others ACT table set (it
contains Square and Copy too), so no table switch lands mid-stream.
"""

import os

import numpy as np

import concourse.bacc as bacc
import concourse.bass as bass
import concourse.mybir as mybir
import concourse.tile as tile
from concourse.bass_utils import run_bass_kernel_spmd

N_CORES = 8
N_TOTAL = 4096
N_LOC = N_TOTAL // N_CORES  # 512 chunks per core
P = 128                     # chunks per tile (partition dim)
T = N_LOC // P              # 4 chunk-tiles per core
D = 128 * 64                # 8192 elements per chunk
S = 10 * 64                 # 640 boundary elements
EPS = 1e-8

F32 = mybir.dt.float32
ALU = mybir.AluOpType
ACTF = mybir.ActivationFunctionType

# --- geometry ---------------------------------------------------------------
GROUPS = [(0, 320), (320, 640)] + [
    (4096 + 512 * g, 4096 + 512 * (g + 1)) for g in range(8)
]                            # 10 bn groups per tile, 20 (e/o) entries
ACT_LO, ACT_HI = S, 4096     # ACT 2-pass region per tile
ACT_MID = 2048               # tile 3 splits its ACT region here
NE = 20                      # bn entries per tile
SLOT = 60                    # bn stats values per tile

# DMA pieces in stream order (tile, elem_lo, elem_hi). Tile 0 ramps with a
# small first piece; tile 3 ends on a single 512 bn-group.
ISSUE = [
    (0, 0, 1024), (0, 1024, 2048), (0, 2048, 4096), (0, 4096, 6144),
    (0, 6144, 8192),
    (1, 0, 2048), (1, 2048, 4096), (1, 4096, 6144), (1, 6144, 8192),
    (2, 0, 2048), (2, 2048, 4096), (2, 4096, 6144), (2, 6144, 8192),
    (3, 0, 2048), (3, 2048, 4096), (3, 4096, 6144), (3, 6144, 7680),
    (3, 7680, 8192),
]


def _sv(t_ap, off, pairs):
    """Strided view of a tile AP: offset + [stride, num] free dims (elems)."""
    ap = t_ap[:] if not isinstance(t_ap, bass.AP) else t_ap
    return bass.AP(
        tensor=ap.tensor,
        offset=ap.offset + off,
        ap=[ap.ap[0], *[[s, n] for s, n in pairs]],
    )


def _build() -> bass.Bass:
    nc = bacc.Bacc(
        "TRN2", target_bir_lowering=False, debug=False, num_devices=N_CORES
    )
    x = nc.dram_tensor("chunks", [N_LOC, 128, 64], F32, kind="ExternalInput")
    ctx_in = nc.dram_tensor("ctx", [S], F32, kind="ExternalInput")
    out = nc.dram_tensor("out", [P, T], F32, kind="ExternalOutput")

    xf = x[:].rearrange("(t p) r f -> t p (r f)", p=P)  # [T, 128, 8192]

    with tile.TileContext(nc) as tc:
        with tc.tile_pool(name="main", bufs=1) as main:
            xts = [
                main.tile([P, D], F32, tag=f"xt{t}", name=f"xt{t}")
                for t in range(T)
            ]
            ctxb = main.tile([P, S], F32)
            bns = main.tile([P, T * SLOT], F32)     # bn stats slots
            sqa = main.tile([P, T], F32)            # ACT Square accums
            suma = main.tile([P, T], F32)           # ACT Copy accums
            sqb = main.tile([P, 1], F32)            # t3 second ACT sub-region
            sumb = main.tile([P, 1], F32)
            nums = main.tile([P, T], F32)           # dot(starts, ctx)
            dump_act = main.tile([P, 1], F32)
            dump_dve = main.tile([P, 1], F32)
            warm = main.tile([P, 1], F32)

            # ---- up-front DMAs: ctx (tiny, gates the dots), then data pieces
            cap = ctx_in[:]
            nc.sync.dma_start(
                out=ctxb,
                in_=bass.AP(tensor=cap.tensor, offset=cap.offset, ap=[[0, P], *cap.ap]),
            )
            for t, lo, hi in ISSUE:
                nc.sync.dma_start(out=xts[t][:, lo:hi], in_=xf[t][:, lo:hi])

            # Pin sqrt_and_others ACT table set before any Square runs.
            nc.vector.memset(warm, 1.0)
            nc.scalar.activation(out=warm, in_=warm, func=ACTF.Sqrt)

            # |ctx|^2 (same value on every partition)
            cn2 = main.tile([P, 1], F32)
            nc.vector.scalar_tensor_tensor(
                out=dump_dve.broadcast_to([P, S]), in0=ctxb, scalar=1.0, in1=ctxb,
                op0=ALU.mult, op1=ALU.mult, accum_out=cn2,
            )

            # ---- helpers
            def bn_group(t, gi, lo, hi):
                nc.vector.bn_stats(
                    out=bns[:, t * SLOT + 6 * gi : t * SLOT + 6 * gi + 6],
                    in_=xts[t][:, lo:hi],
                )

            def act_region(t, lo, hi, sq_slot, sum_slot):
                nc.scalar.activation(
                    out=dump_act.broadcast_to([P, hi - lo]),
                    in_=xts[t][:, lo:hi], func=ACTF.Square, accum_out=sq_slot,
                )
                nc.scalar.activation(
                    out=dump_act.broadcast_to([P, hi - lo]),
                    in_=xts[t][:, lo:hi], func=ACTF.Copy, accum_out=sum_slot,
                )

            # bn stats -> (sum, sumsq, startsq): [c,m,cv] stride-3 views.
            cm = main.tile([P, T * NE], F32)
            s2 = main.tile([P, T * NE], F32)
            sums = main.tile([P, T], F32)
            ssqs = main.tile([P, T], F32)
            startsq = main.tile([P, T], F32)

            def conv_cm(t, e0, e1):
                n = e1 - e0
                off = t * SLOT + 3 * e0
                base = t * NE + e0
                cm_s = cm[:, base : base + n]
                s2_s = s2[:, base : base + n]
                nc.vector.tensor_tensor(
                    out=cm_s, in0=_sv(bns, off, [[3, n]]),
                    in1=_sv(bns, off + 1, [[3, n]]), op=ALU.mult,
                )
                nc.vector.tensor_tensor(
                    out=s2_s, in0=cm_s, in1=_sv(bns, off + 1, [[3, n]]),
                    op=ALU.mult,
                )
                nc.vector.tensor_tensor(
                    out=s2_s, in0=s2_s, in1=_sv(bns, off + 2, [[3, n]]),
                    op=ALU.add,
                )

            # scalar tail tiles
            t0_ = main.tile([P, T], F32)
            v1 = main.tile([P, T], F32)
            std = main.tile([P, T], F32)
            den = main.tile([P, T], F32)
            b1 = main.tile([P, T], F32)
            rec = main.tile([P, T], F32)
            b2 = main.tile([P, T], F32)
            d1 = main.tile([P, T], F32)
            aab = main.tile([P, T], F32)
            b3 = main.tile([P, T], F32)
            m1 = main.tile([P, T], mybir.dt.uint8)
            m2 = main.tile([P, T], mybir.dt.uint8)
            r1 = main.tile([P, T], F32)
            realism = main.tile([P, T], F32)
            den2 = main.tile([P, T], F32)
            rden = main.tile([P, T], F32)
            bnd = main.tile([P, T], F32)
            final = main.tile([P, T], F32)

            def chain(a, b):
                sl = slice(a, b)
                # var*(D-1) = sumsq - sum^2/D ; std = sqrt(v1 / (D-1))
                nc.vector.scalar_tensor_tensor(
                    out=t0_[:, sl], in0=sums[:, sl], scalar=1.0 / D,
                    in1=sums[:, sl], op0=ALU.mult, op1=ALU.mult,
                )
                nc.vector.tensor_tensor(
                    out=v1[:, sl], in0=ssqs[:, sl], in1=t0_[:, sl], op=ALU.subtract
                )
                nc.scalar.activation(
                    out=std[:, sl], in_=v1[:, sl], func=ACTF.Sqrt,
                    scale=1.0 / (D - 1),
                )
                # den = sqrt(startsq * |ctx|^2) on ACT (scale = per-part cn2)
                nc.scalar.activation(
                    out=den[:, sl], in_=startsq[:, sl], func=ACTF.Sqrt, scale=cn2
                )
                # branch values on ACT (Copy computes scale*x + bias)
                nc.scalar.activation(
                    out=b1[:, sl], in_=std[:, sl], func=ACTF.Copy,
                    scale=10.0, bias=0.15,
                )
                nc.vector.reciprocal(out=rec[:, sl], in_=std[:, sl])
                nc.scalar.activation(
                    out=b2[:, sl], in_=rec[:, sl], func=ACTF.Copy,
                    scale=0.5, bias=0.15,
                )
                nc.vector.tensor_scalar(
                    out=d1[:, sl], in0=std[:, sl], scalar1=0.1, scalar2=None,
                    op0=ALU.subtract,
                )
                nc.vector.scalar_tensor_tensor(
                    out=aab[:, sl], in0=d1[:, sl], scalar=-1.0, in1=d1[:, sl],
                    op0=ALU.mult, op1=ALU.max,
                )
                nc.scalar.activation(
                    out=b3[:, sl], in_=aab[:, sl], func=ACTF.Copy,
                    scale=-1.0, bias=1.15,
                )
                nc.vector.tensor_scalar(
                    out=m1[:, sl], in0=std[:, sl], scalar1=0.01, scalar2=None,
                    op0=ALU.is_lt,
                )
                nc.vector.tensor_scalar(
                    out=m2[:, sl], in0=std[:, sl], scalar1=0.5, scalar2=None,
                    op0=ALU.is_gt,
                )
                nc.vector.select(
                    out=r1[:, sl], mask=m2[:, sl], on_true=b2[:, sl],
                    on_false=b3[:, sl],
                )
                nc.vector.select(
                    out=realism[:, sl], mask=m1[:, sl], on_true=b1[:, sl],
                    on_false=r1[:, sl],
                )
                # boundary = num / max(den, eps)
                nc.vector.tensor_scalar(
                    out=den2[:, sl], in0=den[:, sl], scalar1=EPS, scalar2=None,
                    op0=ALU.max,
                )
                nc.vector.reciprocal(out=rden[:, sl], in_=den2[:, sl])
                nc.vector.tensor_tensor(
                    out=bnd[:, sl], in0=nums[:, sl], in1=rden[:, sl], op=ALU.mult
                )
                nc.vector.scalar_tensor_tensor(
                    out=final[:, sl], in0=bnd[:, sl], scalar=0.2,
                    in1=realism[:, sl], op0=ALU.mult, op1=ALU.add,
                )
                nc.sync.dma_start(out=out[:, sl], in_=final[:, sl])

            # ---- per-tile bulk compute + per-tile finish (tiles 0-2)
            for t in range(T):
                nc.vector.scalar_tensor_tensor(
                    out=dump_dve.broadcast_to([P, S]), in0=xts[t][:, :S],
                    scalar=1.0, in1=ctxb, op0=ALU.mult, op1=ALU.mult,
                    accum_out=nums[:, t : t + 1],
                )
                for gi, (lo, hi) in enumerate(GROUPS):
                    bn_group(t, gi, lo, hi)
                if t < T - 1:
                    act_region(t, ACT_LO, ACT_HI,
                               sqa[:, t : t + 1], suma[:, t : t + 1])
                    # full finish for this tile (runs in tile t+1's window)
                    conv_cm(t, 0, NE)
                    nc.vector.tensor_reduce(
                        out=sums[:, t : t + 1], in_=cm[:, t * NE : t * NE + NE],
                        axis=mybir.AxisListType.X, op=ALU.add,
                    )
                    nc.vector.tensor_reduce(
                        out=ssqs[:, t : t + 1], in_=s2[:, t * NE : t * NE + NE],
                        axis=mybir.AxisListType.X, op=ALU.add,
                    )
                    nc.vector.tensor_reduce(
                        out=startsq[:, t : t + 1], in_=s2[:, t * NE : t * NE + 4],
                        axis=mybir.AxisListType.X, op=ALU.add,
                    )
                    nc.vector.tensor_tensor(
                        out=sums[:, t : t + 1], in0=sums[:, t : t + 1],
                        in1=suma[:, t : t + 1], op=ALU.add,
                    )
                    nc.vector.tensor_tensor(
                        out=ssqs[:, t : t + 1], in0=ssqs[:, t : t + 1],
                        in1=sqa[:, t : t + 1], op=ALU.add,
                    )
                    chain(t, t + 1)
                else:
                    # tile 3: split ACT region so both halves fire per piece
                    act_region(t, ACT_LO, ACT_MID,
                               sqa[:, t : t + 1], suma[:, t : t + 1])
                    act_region(t, ACT_MID, ACT_HI, sqb, sumb)

            # ---- tile 3 partials: bn groups 0-8 (entries 0:18) + ACT accums
            t3 = T - 1
            le = t3 * NE
            conv_cm(t3, 0, 18)
            psum3 = main.tile([P, 1], F32)
            pssq3 = main.tile([P, 1], F32)
            acts3 = main.tile([P, 2], F32)
            nc.vector.tensor_reduce(
                out=psum3, in_=cm[:, le : le + 18],
                axis=mybir.AxisListType.X, op=ALU.add,
            )
            nc.vector.tensor_reduce(
                out=pssq3, in_=s2[:, le : le + 18],
                axis=mybir.AxisListType.X, op=ALU.add,
            )
            nc.vector.tensor_reduce(
                out=startsq[:, 3:4], in_=s2[:, le : le + 4],
                axis=mybir.AxisListType.X, op=ALU.add,
            )
            nc.vector.tensor_tensor(
                out=acts3[:, 0:1], in0=suma[:, 3:4], in1=sumb, op=ALU.add
            )
            nc.vector.tensor_tensor(
                out=acts3[:, 1:2], in0=sqa[:, 3:4], in1=sqb, op=ALU.add
            )
            nc.vector.tensor_tensor(out=psum3, in0=psum3, in1=acts3[:, 0:1], op=ALU.add)
            nc.vector.tensor_tensor(out=pssq3, in0=pssq3, in1=acts3[:, 1:2], op=ALU.add)

            # tile 3 final 512-group (the only post-stream stats work)
            conv_cm(t3, 18, 20)
            t9 = main.tile([P, 2], F32)
            nc.vector.tensor_reduce(
                out=t9[:, 0:1], in_=cm[:, le + 18 : le + 20],
                axis=mybir.AxisListType.X, op=ALU.add,
            )
            nc.vector.tensor_reduce(
                out=t9[:, 1:2], in_=s2[:, le + 18 : le + 20],
                axis=mybir.AxisListType.X, op=ALU.add,
            )
            nc.vector.tensor_tensor(out=sums[:, 3:4], in0=psum3, in1=t9[:, 0:1], op=ALU.add)
            nc.vector.tensor_tensor(out=ssqs[:, 3:4], in0=pssq3, in1=t9[:, 1:2], op=ALU.add)
            chain(3, 4)
    nc.compile()
    return nc


_NC_CACHE = None


def _get_nc() -> bass.Bass:
    global _NC_CACHE
    if _NC_CACHE is None:
        _NC_CACHE = _build()
    return _NC_CACHE


def run(inputs: dict, trace: bool = False, **kw):
    """Returns (output [4096] f32, BassKernelResults)."""
    chunks = np.ascontiguousarray(np.asarray(inputs["chunks"], dtype=np.float32))
    pc = np.asarray(inputs["previous_context"], dtype=np.float32)
    ctx = np.ascontiguousarray(pc[-10:].reshape(-1))
    assert chunks.shape == (N_TOTAL, 128, 64)
    assert ctx.shape == (S,)

    nc = _get_nc()
    in_maps = [
        {"chunks": chunks[c * N_LOC : (c + 1) * N_LOC], "ctx": ctx}
        for c in range(N_CORES)
    ]
    res = run_bass_kernel_spmd(nc, in_maps, core_ids=list(range(N_CORES)),
                               trace=trace, **kw)
    # out[p, t] = score of local chunk t*128+p -> transpose to chunk order
    full = np.concatenate([r["out"].T.reshape(-1) for r in res.results])
    return full.astype(np.float32), res


def kernel(**inputs) -> np.ndarray:
    return run(inputs)[0]


# revision 13
# speedup vs baseline: 1.0069x; 1.0069x over previous
"""ChunkRanker Bass kernel for Trainium2, 8-core data-parallel.

Math per chunk n (chunks: [4096, 128, 64] f32):
  flat = chunks[n].reshape(8192)
  std  = std(flat, ddof=1)
  realism = std<0.01 ? 10*std : (std>0.5 ? 0.5/std : 1-|std-0.1|)
  ctx    = previous_context[-10:].flatten()            # [640]
  starts = flat[:640]
  boundary = dot(starts, ctx) / max(|starts|*|ctx|, 1e-8)
  score = realism + 0.3*0.5 + 0.2*boundary

Sharding: leading chunk axis split 8 ways (512 chunks/core); ctx broadcast.

Per-core layout: 4 static SBUF tiles of [128 partitions = chunks, 8192 free =
chunk elements] (16 MB -- the whole local input stays resident, so every DMA
is enqueued up front and the 16 queues stream at full rate, ~441 GB/s, with
no buffer-reuse stalls). Tiles are DMAed in ~0.5-1 MB pieces so compute
tracks the stream.

Reduction split (std needs sum AND sumsq = 2 moments per chunk):
  - DVE bn_stats yields BOTH moments in one pass (593ns/512-group, i.e.
    0.86 elem/ns); boundary [0,640) runs as 2x320 groups whose stats also
    give |starts|^2 for free; dot(starts,ctx) via STT accum.
  - ACT covers a contiguous region with two accumulating passes (Square then
    Copy, 0.54 elem/ns combined) -- both moments, one engine.
Tiles 0-2: ACT region [640,4096), bn 8x512 over [4096,8192).
Tile 3 (streamed last): its data alternates small bn pieces and small ACT
sub-regions sized to each engine's rate (61:39), so BOTH engines follow the
stream tail concurrently and neither accumulates a backlog; the last ACT
sub-region is small (512) and early so ACT drains well before the stream
ends. The tiles-0-2 stats conversion + scalar chain runs in the idle window
before tile 3's data begins arriving. Tile 3's bn groups 0-9 fold into
partials mid-stream (the ACT-accum adds are a separate step gated only on
ACT), so after the final 512-group lands only [1 bn_stats + tiny combines +
a [128,1] scalar chain + 2KB out-DMA] remain.

bn group stats [c,m,c*var] convert to (sum, sumsq): sum_g = c*m,
ssq_g = c*var + (c*m)*m.

A dummy sqrt at kernel start pins the sqrt_and_others ACT table set (it
contains Square and Copy too), so no table switch lands mid-stream.
"""

import os

import numpy as np

import concourse.bacc as bacc
import concourse.bass as bass
import concourse.mybir as mybir
import concourse.tile as tile
from concourse.bass_utils import run_bass_kernel_spmd

N_CORES = 8
N_TOTAL = 4096
N_LOC = N_TOTAL // N_CORES  # 512 chunks per core
P = 128                     # chunks per tile (partition dim)
T = N_LOC // P              # 4 chunk-tiles per core
D = 128 * 64                # 8192 elements per chunk
S = 10 * 64                 # 640 boundary elements
EPS = 1e-8

# Scheduler hint knob (sim-ms floor for the tiles-0-2 chain block); 0 = off.
CHAIN03_WAIT_MS = float(os.environ.get("K_CHAIN03_WAIT", "0.0"))

F32 = mybir.dt.float32
ALU = mybir.AluOpType
ACTF = mybir.ActivationFunctionType

# --- geometry ---------------------------------------------------------------
# Tiles 0-2 ("E" tiles): bn groups 2x320 + 8x512, ACT region [640, 4096).
E_GROUPS = [(0, 320), (320, 640)] + [
    (4096 + 512 * g, 4096 + 512 * (g + 1)) for g in range(8)
]
E_ACT = (S, 4096)
# Tile 3 ("L" tile, streamed last): 2x320 + 9x512 bn groups alternating with
# three ACT sub-regions (the last one small + early so ACT drains first).
L_GROUPS = [
    (0, 320), (320, 640), (640, 1152), (1152, 1664),
    (2816, 3328), (3328, 3840), (5120, 5632), (6144, 6656),
    (6656, 7168), (7168, 7680), (7680, 8192),
]
L_ACTS = [(1664, 2816), (3840, 5120), (5632, 6144)]
SLOT = 66                   # bn stats slots per tile (11 groups max * 6)
E_NE = 20                   # bn entries per E tile (10 groups * 2)
L_NE = 22                   # bn entries for the L tile
CMB = [0, E_NE, 2 * E_NE, 3 * E_NE]   # compact cm/s2 base per tile

# DMA pieces in stream order (tile, elem_lo, elem_hi).
ISSUE = [
    (0, 0, 1024), (0, 1024, 2048), (0, 2048, 4096), (0, 4096, 6144),
    (0, 6144, 8192),
    (1, 0, 2048), (1, 2048, 4096), (1, 4096, 6144), (1, 6144, 8192),
    (2, 0, 2048), (2, 2048, 4096), (2, 4096, 6144), (2, 6144, 8192),
    (3, 0, 1664), (3, 1664, 2816), (3, 2816, 3840), (3, 3840, 5120),
    (3, 5120, 6144), (3, 6144, 7680), (3, 7680, 8192),
]


def _sv(t_ap, off, pairs):
    """Strided view of a tile AP: offset + [stride, num] free dims (elems)."""
    ap = t_ap[:] if not isinstance(t_ap, bass.AP) else t_ap
    return bass.AP(
        tensor=ap.tensor,
        offset=ap.offset + off,
        ap=[ap.ap[0], *[[s, n] for s, n in pairs]],
    )


def _build() -> bass.Bass:
    nc = bacc.Bacc(
        "TRN2", target_bir_lowering=False, debug=False, num_devices=N_CORES
    )
    x = nc.dram_tensor("chunks", [N_LOC, 128, 64], F32, kind="ExternalInput")
    ctx_in = nc.dram_tensor("ctx", [S], F32, kind="ExternalInput")
    out = nc.dram_tensor("out", [P, T], F32, kind="ExternalOutput")

    xf = x[:].rearrange("(t p) r f -> t p (r f)", p=P)  # [T, 128, 8192]

    with tile.TileContext(nc) as tc:
        with tc.tile_pool(name="main", bufs=1) as main:
            xts = [
                main.tile([P, D], F32, tag=f"xt{t}", name=f"xt{t}")
                for t in range(T)
            ]
            ctxb = main.tile([P, S], F32)
            bns = main.tile([P, T * SLOT], F32)     # bn stats slots
            sqa = main.tile([P, T], F32)            # ACT Square accums (E tiles)
            suma = main.tile([P, T], F32)           # ACT Copy accums
            sqb = main.tile([P, 3], F32)            # L tile ACT sub-regions
            sumb = main.tile([P, 3], F32)
            nums = main.tile([P, T], F32)           # dot(starts, ctx)
            dump_act = main.tile([P, 1], F32)
            dump_dve = main.tile([P, 1], F32)
            warm = main.tile([P, 1], F32)

            # ---- up-front DMAs: ctx (tiny, gates the dots), then data pieces
            cap = ctx_in[:]
            nc.sync.dma_start(
                out=ctxb,
                in_=bass.AP(tensor=cap.tensor, offset=cap.offset, ap=[[0, P], *cap.ap]),
            )
            for t, lo, hi in ISSUE:
                nc.sync.dma_start(out=xts[t][:, lo:hi], in_=xf[t][:, lo:hi])

            # Pin sqrt_and_others ACT table set before any Square runs.
            nc.vector.memset(warm, 1.0)
            nc.scalar.activation(out=warm, in_=warm, func=ACTF.Sqrt)

            # |ctx|^2 (same value on every partition)
            cn2 = main.tile([P, 1], F32)
            nc.vector.scalar_tensor_tensor(
                out=dump_dve.broadcast_to([P, S]), in0=ctxb, scalar=1.0, in1=ctxb,
                op0=ALU.mult, op1=ALU.mult, accum_out=cn2,
            )

            # ---- per-tile bulk compute
            def bn_group(t, gi, lo, hi):
                nc.vector.bn_stats(
                    out=bns[:, t * SLOT + 6 * gi : t * SLOT + 6 * gi + 6],
                    in_=xts[t][:, lo:hi],
                )

            def act_region(t, lo, hi, sq_slot, sum_slot):
                nc.scalar.activation(
                    out=dump_act.broadcast_to([P, hi - lo]),
                    in_=xts[t][:, lo:hi], func=ACTF.Square, accum_out=sq_slot,
                )
                nc.scalar.activation(
                    out=dump_act.broadcast_to([P, hi - lo]),
                    in_=xts[t][:, lo:hi], func=ACTF.Copy, accum_out=sum_slot,
                )

            for t in range(T):
                nc.vector.scalar_tensor_tensor(
                    out=dump_dve.broadcast_to([P, S]), in0=xts[t][:, :S],
                    scalar=1.0, in1=ctxb, op0=ALU.mult, op1=ALU.mult,
                    accum_out=nums[:, t : t + 1],
                )
                groups = E_GROUPS if t < T - 1 else L_GROUPS
                for gi, (lo, hi) in enumerate(groups):
                    bn_group(t, gi, lo, hi)
                if t < T - 1:
                    act_region(t, *E_ACT, sqa[:, t : t + 1], suma[:, t : t + 1])
                else:
                    for k, (lo, hi) in enumerate(L_ACTS):
                        act_region(t, lo, hi, sqb[:, k : k + 1], sumb[:, k : k + 1])

            # ---- bn stats -> (sum, sumsq, startsq) conversion
            # per-group stats [ce,me,cve,co,mo,cvo]; stride-3 views give
            # c/m/cv streams in (g,e/o) order; entries 0:4 = boundary.
            cm = main.tile([P, 3 * E_NE + L_NE], F32)
            s2 = main.tile([P, 3 * E_NE + L_NE], F32)
            sums = main.tile([P, T], F32)
            ssqs = main.tile([P, T], F32)
            startsq = main.tile([P, T], F32)

            def conv_cm(c_v, m_v, cv_v, base, n):
                cm_s = cm[:, base : base + n]
                s2_s = s2[:, base : base + n]
                nc.vector.tensor_tensor(out=cm_s, in0=c_v, in1=m_v, op=ALU.mult)
                nc.vector.tensor_tensor(out=s2_s, in0=cm_s, in1=m_v, op=ALU.mult)
                nc.vector.tensor_tensor(out=s2_s, in0=s2_s, in1=cv_v, op=ALU.add)

            import contextlib
            w03 = (
                tc.tile_wait_until(CHAIN03_WAIT_MS)
                if CHAIN03_WAIT_MS > 0 else contextlib.nullcontext()
            )
            with w03:
                # tiles 0-2 batched: 3-dim strided views [tile, entry]
                conv_cm(
                    _sv(bns, 0, [[SLOT, 3], [3, E_NE]]),
                    _sv(bns, 1, [[SLOT, 3], [3, E_NE]]),
                    _sv(bns, 2, [[SLOT, 3], [3, E_NE]]),
                    0, 3 * E_NE,
                )
                nc.vector.tensor_reduce(
                    out=sums[:, 0:3],
                    in_=cm[:, : 3 * E_NE].rearrange("p (t k) -> p t k", k=E_NE),
                    axis=mybir.AxisListType.X, op=ALU.add,
                )
                nc.vector.tensor_reduce(
                    out=ssqs[:, 0:3],
                    in_=s2[:, : 3 * E_NE].rearrange("p (t k) -> p t k", k=E_NE),
                    axis=mybir.AxisListType.X, op=ALU.add,
                )
                nc.vector.tensor_reduce(
                    out=startsq[:, 0:3],
                    in_=_sv(s2, 0, [[E_NE, 3], [1, 4]]),
                    axis=mybir.AxisListType.X, op=ALU.add,
                )
                nc.vector.tensor_tensor(
                    out=sums[:, 0:3], in0=sums[:, 0:3], in1=suma[:, 0:3], op=ALU.add
                )
                nc.vector.tensor_tensor(
                    out=ssqs[:, 0:3], in0=ssqs[:, 0:3], in1=sqa[:, 0:3], op=ALU.add
                )

            # L-tile bn partials: entries 0:20 (groups 0-9) fold mid-stream.
            lb = 3 * SLOT
            le = CMB[3]
            conv_cm(
                _sv(bns, lb, [[3, 20]]),
                _sv(bns, lb + 1, [[3, 20]]),
                _sv(bns, lb + 2, [[3, 20]]),
                le, 20,
            )
            psum3 = main.tile([P, 1], F32)
            pssq3 = main.tile([P, 1], F32)
            nc.vector.tensor_reduce(
                out=psum3, in_=cm[:, le : le + 20],
                axis=mybir.AxisListType.X, op=ALU.add,
            )
            nc.vector.tensor_reduce(
                out=pssq3, in_=s2[:, le : le + 20],
                axis=mybir.AxisListType.X, op=ALU.add,
            )
            nc.vector.tensor_reduce(
                out=startsq[:, 3:4], in_=s2[:, le : le + 4],
                axis=mybir.AxisListType.X, op=ALU.add,
            )
            # ACT-accum partial (gated only on ACT's sub-regions)
            acts3 = main.tile([P, 2], F32)
            nc.vector.tensor_reduce(
                out=acts3[:, 0:1], in_=sumb, axis=mybir.AxisListType.X, op=ALU.add
            )
            nc.vector.tensor_reduce(
                out=acts3[:, 1:2], in_=sqb, axis=mybir.AxisListType.X, op=ALU.add
            )
            nc.vector.tensor_tensor(out=psum3, in0=psum3, in1=acts3[:, 0:1], op=ALU.add)
            nc.vector.tensor_tensor(out=pssq3, in0=pssq3, in1=acts3[:, 1:2], op=ALU.add)

            # L-tile final 512-group (the only post-stream stats work)
            conv_cm(
                _sv(bns, lb + 60, [[3, 2]]),
                _sv(bns, lb + 61, [[3, 2]]),
                _sv(bns, lb + 62, [[3, 2]]),
                le + 20, 2,
            )
            t9 = main.tile([P, 2], F32)
            nc.vector.tensor_reduce(
                out=t9[:, 0:1], in_=cm[:, le + 20 : le + 22],
                axis=mybir.AxisListType.X, op=ALU.add,
            )
            nc.vector.tensor_reduce(
                out=t9[:, 1:2], in_=s2[:, le + 20 : le + 22],
                axis=mybir.AxisListType.X, op=ALU.add,
            )
            nc.vector.tensor_tensor(out=sums[:, 3:4], in0=psum3, in1=t9[:, 0:1], op=ALU.add)
            nc.vector.tensor_tensor(out=ssqs[:, 3:4], in0=pssq3, in1=t9[:, 1:2], op=ALU.add)

            # ---- scalar tail, run per column-batch (0:3 early, 3:4 late)
            t0_ = main.tile([P, T], F32)
            v1 = main.tile([P, T], F32)
            std = main.tile([P, T], F32)
            den = main.tile([P, T], F32)
            b1 = main.tile([P, T], F32)
            rec = main.tile([P, T], F32)
            b2 = main.tile([P, T], F32)
            d1 = main.tile([P, T], F32)
            aab = main.tile([P, T], F32)
            b3 = main.tile([P, T], F32)
            m1 = main.tile([P, T], mybir.dt.uint8)
            m2 = main.tile([P, T], mybir.dt.uint8)
            r1 = main.tile([P, T], F32)
            realism = main.tile([P, T], F32)
            den2 = main.tile([P, T], F32)
            rden = main.tile([P, T], F32)
            bnd = main.tile([P, T], F32)
            final = main.tile([P, T], F32)

            def chain(a, b):
                sl = slice(a, b)
                # var*(D-1) = sumsq - sum^2/D ; std = sqrt(v1 / (D-1))
                nc.vector.scalar_tensor_tensor(
                    out=t0_[:, sl], in0=sums[:, sl], scalar=1.0 / D,
                    in1=sums[:, sl], op0=ALU.mult, op1=ALU.mult,
                )
                nc.vector.tensor_tensor(
                    out=v1[:, sl], in0=ssqs[:, sl], in1=t0_[:, sl], op=ALU.subtract
                )
                nc.scalar.activation(
                    out=std[:, sl], in_=v1[:, sl], func=ACTF.Sqrt,
                    scale=1.0 / (D - 1),
                )
                # den = sqrt(startsq * |ctx|^2) on ACT (scale = per-part cn2)
                nc.scalar.activation(
                    out=den[:, sl], in_=startsq[:, sl], func=ACTF.Sqrt, scale=cn2
                )
                # piecewise realism (+0.15 regime term folded in)
                nc.vector.tensor_scalar(
                    out=b1[:, sl], in0=std[:, sl], scalar1=10.0, scalar2=0.15,
                    op0=ALU.mult, op1=ALU.add,
                )
                nc.vector.reciprocal(out=rec[:, sl], in_=std[:, sl])
                nc.vector.tensor_scalar(
                    out=b2[:, sl], in0=rec[:, sl], scalar1=0.5, scalar2=0.15,
                    op0=ALU.mult, op1=ALU.add,
                )
                nc.vector.tensor_scalar(
                    out=d1[:, sl], in0=std[:, sl], scalar1=0.1, scalar2=None,
                    op0=ALU.subtract,
                )
                nc.vector.scalar_tensor_tensor(
                    out=aab[:, sl], in0=d1[:, sl], scalar=-1.0, in1=d1[:, sl],
                    op0=ALU.mult, op1=ALU.max,
                )
                nc.vector.tensor_scalar(
                    out=b3[:, sl], in0=aab[:, sl], scalar1=-1.0, scalar2=1.15,
                    op0=ALU.mult, op1=ALU.add,
                )
                nc.vector.tensor_scalar(
                    out=m1[:, sl], in0=std[:, sl], scalar1=0.01, scalar2=None,
                    op0=ALU.is_lt,
                )
                nc.vector.tensor_scalar(
                    out=m2[:, sl], in0=std[:, sl], scalar1=0.5, scalar2=None,
                    op0=ALU.is_gt,
                )
                nc.vector.select(
                    out=r1[:, sl], mask=m2[:, sl], on_true=b2[:, sl],
                    on_false=b3[:, sl],
                )
                nc.vector.select(
                    out=realism[:, sl], mask=m1[:, sl], on_true=b1[:, sl],
                    on_false=r1[:, sl],
                )
                # boundary = num / max(den, eps)
                nc.vector.tensor_scalar(
                    out=den2[:, sl], in0=den[:, sl], scalar1=EPS, scalar2=None,
                    op0=ALU.max,
                )
                nc.vector.reciprocal(out=rden[:, sl], in_=den2[:, sl])
                nc.vector.tensor_tensor(
                    out=bnd[:, sl], in0=nums[:, sl], in1=rden[:, sl], op=ALU.mult
                )
                nc.vector.scalar_tensor_tensor(
                    out=final[:, sl], in0=bnd[:, sl], scalar=0.2,
                    in1=realism[:, sl], op0=ALU.mult, op1=ALU.add,
                )

            with w03 if CHAIN03_WAIT_MS <= 0 else tc.tile_wait_until(CHAIN03_WAIT_MS):
                chain(0, 3)
                nc.sync.dma_start(out=out[:, 0:3], in_=final[:, 0:3])
            chain(3, 4)
            nc.sync.dma_start(out=out[:, 3:4], in_=final[:, 3:4])
    nc.compile()
    return nc


_NC_CACHE = None


def _get_nc() -> bass.Bass:
    global _NC_CACHE
    if _NC_CACHE is None:
        _NC_CACHE = _build()
    return _NC_CACHE


def run(inputs: dict, trace: bool = False, **kw):
    """Returns (output [4096] f32, BassKernelResults)."""
    chunks = np.ascontiguousarray(np.asarray(inputs["chunks"], dtype=np.float32))
    pc = np.asarray(inputs["previous_context"], dtype=np.float32)
    ctx = np.ascontiguousarray(pc[-10:].reshape(-1))
    assert chunks.shape == (N_TOTAL, 128, 64)
    assert ctx.shape == (S,)

    nc = _get_nc()
    in_maps = [
        {"chunks": chunks[c * N_LOC : (c + 1) * N_LOC], "ctx": ctx}
        for c in range(N_CORES)
    ]
    res = run_bass_kernel_spmd(nc, in_maps, core_ids=list(range(N_CORES)),
                               trace=trace, **kw)
    # out[p, t] = score of local chunk t*128+p -> transpose to chunk order
    full = np.concatenate([r["out"].T.reshape(-1) for r in res.results])
    return full.astype(np.float32), res


def kernel(**inputs) -> np.ndarray:
    return run(inputs)[0]


# revision 14
# speedup vs baseline: 1.0404x; 1.0333x over previous
"""ChunkRanker Bass kernel for Trainium2, 8-core data-parallel.

Math per chunk n (chunks: [4096, 128, 64] f32):
  flat = chunks[n].reshape(8192)
  std  = std(flat, ddof=1)
  realism = std<0.01 ? 10*std : (std>0.5 ? 0.5/std : 1-|std-0.1|)
  ctx    = previous_context[-10:].flatten()            # [640]
  starts = flat[:640]
  boundary = dot(starts, ctx) / max(|starts|*|ctx|, 1e-8)
  score = realism + 0.3*0.5 + 0.2*boundary

Sharding: leading chunk axis split 8 ways (512 chunks/core); ctx broadcast.

Per-core layout: 4 static SBUF tiles of [128 partitions = chunks, 8192 free =
chunk elements] (16 MB -- the whole local input stays resident, so every DMA
is enqueued up front and the 16 queues stream at full rate, ~441 GB/s, with
no buffer-reuse stalls). Tiles are DMAed in ~0.5-1 MB pieces so compute
tracks the stream.

Reduction split (std needs sum AND sumsq = 2 moments per chunk):
  - DVE bn_stats yields BOTH moments in one pass (593ns/512-group, i.e.
    0.86 elem/ns); boundary [0,640) runs as 2x320 groups whose stats also
    give |starts|^2 for free; dot(starts,ctx) via STT accum.
  - ACT covers a contiguous region with two accumulating passes (Square then
    Copy, 0.54 elem/ns combined) -- both moments, one engine.
Tiles 0-2: ACT region [640,4096), bn 8x512 over [4096,8192).
Tile 3 (streamed last): its data alternates small bn pieces and small ACT
sub-regions sized to each engine's rate (61:39), so BOTH engines follow the
stream tail concurrently and neither accumulates a backlog; the last ACT
sub-region is small (512) and early so ACT drains well before the stream
ends. The tiles-0-2 stats conversion + scalar chain runs in the idle window
before tile 3's data begins arriving. Tile 3's bn groups 0-9 fold into
partials mid-stream (the ACT-accum adds are a separate step gated only on
ACT), so after the final 512-group lands only [1 bn_stats + tiny combines +
a [128,1] scalar chain + 2KB out-DMA] remain.

bn group stats [c,m,c*var] convert to (sum, sumsq): sum_g = c*m,
ssq_g = c*var + (c*m)*m.

A dummy sqrt at kernel start pins the sqrt_and_others ACT table set (it
contains Square and Copy too), so no table switch lands mid-stream.
"""

import os

import numpy as np

import concourse.bacc as bacc
import concourse.bass as bass
import concourse.mybir as mybir
import concourse.tile as tile
from concourse.bass_utils import run_bass_kernel_spmd

N_CORES = 8
N_TOTAL = 4096
N_LOC = N_TOTAL // N_CORES  # 512 chunks per core
P = 128                     # chunks per tile (partition dim)
T = N_LOC // P              # 4 chunk-tiles per core
D = 128 * 64                # 8192 elements per chunk
S = 10 * 64                 # 640 boundary elements
EPS = 1e-8

# Scheduler hint knob (sim-ms floor for the tiles-0-2 chain block); 0 = off.
CHAIN03_WAIT_MS = float(os.environ.get("K_CHAIN03_WAIT", "0.0"))

F32 = mybir.dt.float32
ALU = mybir.AluOpType
ACTF = mybir.ActivationFunctionType

# --- geometry ---------------------------------------------------------------
# Tiles 0-2 ("E" tiles): bn groups 2x320 + 8x512, ACT region [640, 4096).
E_GROUPS = [(0, 320), (320, 640)] + [
    (4096 + 512 * g, 4096 + 512 * (g + 1)) for g in range(8)
]
E_ACT = (S, 4096)
# Tile 3 ("L" tile, streamed last): 2x320 + 9x512 bn groups alternating with
# three ACT sub-regions (the last one small + early so ACT drains first).
L_GROUPS = [
    (0, 320), (320, 640), (640, 1152), (1152, 1664),
    (2624, 3136), (3136, 3648), (4640, 5152), (5152, 5664),
    (6656, 7168), (7168, 7680), (7680, 8192),
]
L_ACTS = [(1664, 2624), (3648, 4640), (5664, 6656)]
SLOT = 66                   # bn stats slots per tile (11 groups max * 6)
E_NE = 20                   # bn entries per E tile (10 groups * 2)
L_NE = 22                   # bn entries for the L tile
CMB = [0, E_NE, 2 * E_NE, 3 * E_NE]   # compact cm/s2 base per tile

# DMA pieces in stream order (tile, elem_lo, elem_hi).
ISSUE = [
    (0, 0, 1024), (0, 1024, 2048), (0, 2048, 4096), (0, 4096, 6144),
    (0, 6144, 8192),
    (1, 0, 2048), (1, 2048, 4096), (1, 4096, 6144), (1, 6144, 8192),
    (2, 0, 2048), (2, 2048, 4096), (2, 4096, 6144), (2, 6144, 8192),
    (3, 0, 1664), (3, 1664, 2624), (3, 2624, 3648), (3, 3648, 4640),
    (3, 4640, 5664), (3, 5664, 6656), (3, 6656, 7680), (3, 7680, 8192),
]


def _sv(t_ap, off, pairs):
    """Strided view of a tile AP: offset + [stride, num] free dims (elems)."""
    ap = t_ap[:] if not isinstance(t_ap, bass.AP) else t_ap
    return bass.AP(
        tensor=ap.tensor,
        offset=ap.offset + off,
        ap=[ap.ap[0], *[[s, n] for s, n in pairs]],
    )


def _build() -> bass.Bass:
    nc = bacc.Bacc(
        "TRN2", target_bir_lowering=False, debug=False, num_devices=N_CORES
    )
    x = nc.dram_tensor("chunks", [N_LOC, 128, 64], F32, kind="ExternalInput")
    ctx_in = nc.dram_tensor("ctx", [S], F32, kind="ExternalInput")
    out = nc.dram_tensor("out", [P, T], F32, kind="ExternalOutput")

    xf = x[:].rearrange("(t p) r f -> t p (r f)", p=P)  # [T, 128, 8192]

    with tile.TileContext(nc) as tc:
        with tc.tile_pool(name="main", bufs=1) as main:
            xts = [
                main.tile([P, D], F32, tag=f"xt{t}", name=f"xt{t}")
                for t in range(T)
            ]
            ctxb = main.tile([P, S], F32)
            bns = main.tile([P, T * SLOT], F32)     # bn stats slots
            sqa = main.tile([P, T], F32)            # ACT Square accums (E tiles)
            suma = main.tile([P, T], F32)           # ACT Copy accums
            sqb = main.tile([P, 3], F32)            # L tile ACT sub-regions
            sumb = main.tile([P, 3], F32)
            nums = main.tile([P, T], F32)           # dot(starts, ctx)
            dump_act = main.tile([P, 1], F32)
            dump_dve = main.tile([P, 1], F32)
            warm = main.tile([P, 1], F32)

            # ---- up-front DMAs: ctx (tiny, gates the dots), then data pieces
            cap = ctx_in[:]
            nc.sync.dma_start(
                out=ctxb,
                in_=bass.AP(tensor=cap.tensor, offset=cap.offset, ap=[[0, P], *cap.ap]),
            )
            for t, lo, hi in ISSUE:
                nc.sync.dma_start(out=xts[t][:, lo:hi], in_=xf[t][:, lo:hi])

            # Pin sqrt_and_others ACT table set before any Square runs.
            nc.vector.memset(warm, 1.0)
            nc.scalar.activation(out=warm, in_=warm, func=ACTF.Sqrt)

            # |ctx|^2 (same value on every partition)
            cn2 = main.tile([P, 1], F32)
            nc.vector.scalar_tensor_tensor(
                out=dump_dve.broadcast_to([P, S]), in0=ctxb, scalar=1.0, in1=ctxb,
                op0=ALU.mult, op1=ALU.mult, accum_out=cn2,
            )

            # ---- per-tile bulk compute
            def bn_group(t, gi, lo, hi):
                nc.vector.bn_stats(
                    out=bns[:, t * SLOT + 6 * gi : t * SLOT + 6 * gi + 6],
                    in_=xts[t][:, lo:hi],
                )

            def act_region(t, lo, hi, sq_slot, sum_slot):
                nc.scalar.activation(
                    out=dump_act.broadcast_to([P, hi - lo]),
                    in_=xts[t][:, lo:hi], func=ACTF.Square, accum_out=sq_slot,
                )
                nc.scalar.activation(
                    out=dump_act.broadcast_to([P, hi - lo]),
                    in_=xts[t][:, lo:hi], func=ACTF.Copy, accum_out=sum_slot,
                )

            for t in range(T):
                nc.vector.scalar_tensor_tensor(
                    out=dump_dve.broadcast_to([P, S]), in0=xts[t][:, :S],
                    scalar=1.0, in1=ctxb, op0=ALU.mult, op1=ALU.mult,
                    accum_out=nums[:, t : t + 1],
                )
                groups = E_GROUPS if t < T - 1 else L_GROUPS
                for gi, (lo, hi) in enumerate(groups):
                    bn_group(t, gi, lo, hi)
                if t < T - 1:
                    act_region(t, *E_ACT, sqa[:, t : t + 1], suma[:, t : t + 1])
                else:
                    for k, (lo, hi) in enumerate(L_ACTS):
                        act_region(t, lo, hi, sqb[:, k : k + 1], sumb[:, k : k + 1])

            # ---- bn stats -> (sum, sumsq, startsq) conversion
            # per-group stats [ce,me,cve,co,mo,cvo]; stride-3 views give
            # c/m/cv streams in (g,e/o) order; entries 0:4 = boundary.
            cm = main.tile([P, 3 * E_NE + L_NE], F32)
            s2 = main.tile([P, 3 * E_NE + L_NE], F32)
            sums = main.tile([P, T], F32)
            ssqs = main.tile([P, T], F32)
            startsq = main.tile([P, T], F32)

            def conv_cm(c_v, m_v, cv_v, base, n):
                cm_s = cm[:, base : base + n]
                s2_s = s2[:, base : base + n]
                nc.vector.tensor_tensor(out=cm_s, in0=c_v, in1=m_v, op=ALU.mult)
                nc.vector.tensor_tensor(out=s2_s, in0=cm_s, in1=m_v, op=ALU.mult)
                nc.vector.tensor_tensor(out=s2_s, in0=s2_s, in1=cv_v, op=ALU.add)

            import contextlib
            w03 = (
                tc.tile_wait_until(CHAIN03_WAIT_MS)
                if CHAIN03_WAIT_MS > 0 else contextlib.nullcontext()
            )
            with w03:
                # tiles 0-2 batched: 3-dim strided views [tile, entry]
                conv_cm(
                    _sv(bns, 0, [[SLOT, 3], [3, E_NE]]),
                    _sv(bns, 1, [[SLOT, 3], [3, E_NE]]),
                    _sv(bns, 2, [[SLOT, 3], [3, E_NE]]),
                    0, 3 * E_NE,
                )
                nc.vector.tensor_reduce(
                    out=sums[:, 0:3],
                    in_=cm[:, : 3 * E_NE].rearrange("p (t k) -> p t k", k=E_NE),
                    axis=mybir.AxisListType.X, op=ALU.add,
                )
                nc.vector.tensor_reduce(
                    out=ssqs[:, 0:3],
                    in_=s2[:, : 3 * E_NE].rearrange("p (t k) -> p t k", k=E_NE),
                    axis=mybir.AxisListType.X, op=ALU.add,
                )
                nc.vector.tensor_reduce(
                    out=startsq[:, 0:3],
                    in_=_sv(s2, 0, [[E_NE, 3], [1, 4]]),
                    axis=mybir.AxisListType.X, op=ALU.add,
                )
                nc.vector.tensor_tensor(
                    out=sums[:, 0:3], in0=sums[:, 0:3], in1=suma[:, 0:3], op=ALU.add
                )
                nc.vector.tensor_tensor(
                    out=ssqs[:, 0:3], in0=ssqs[:, 0:3], in1=sqa[:, 0:3], op=ALU.add
                )

            # L-tile bn partials: entries 0:20 (groups 0-9) fold mid-stream.
            lb = 3 * SLOT
            le = CMB[3]
            conv_cm(
                _sv(bns, lb, [[3, 20]]),
                _sv(bns, lb + 1, [[3, 20]]),
                _sv(bns, lb + 2, [[3, 20]]),
                le, 20,
            )
            psum3 = main.tile([P, 1], F32)
            pssq3 = main.tile([P, 1], F32)
            nc.vector.tensor_reduce(
                out=psum3, in_=cm[:, le : le + 20],
                axis=mybir.AxisListType.X, op=ALU.add,
            )
            nc.vector.tensor_reduce(
                out=pssq3, in_=s2[:, le : le + 20],
                axis=mybir.AxisListType.X, op=ALU.add,
            )
            nc.vector.tensor_reduce(
                out=startsq[:, 3:4], in_=s2[:, le : le + 4],
                axis=mybir.AxisListType.X, op=ALU.add,
            )
            # ACT-accum partial (gated only on ACT's sub-regions)
            acts3 = main.tile([P, 2], F32)
            nc.vector.tensor_reduce(
                out=acts3[:, 0:1], in_=sumb, axis=mybir.AxisListType.X, op=ALU.add
            )
            nc.vector.tensor_reduce(
                out=acts3[:, 1:2], in_=sqb, axis=mybir.AxisListType.X, op=ALU.add
            )
            nc.vector.tensor_tensor(out=psum3, in0=psum3, in1=acts3[:, 0:1], op=ALU.add)
            nc.vector.tensor_tensor(out=pssq3, in0=pssq3, in1=acts3[:, 1:2], op=ALU.add)

            # L-tile final 512-group (the only post-stream stats work)
            conv_cm(
                _sv(bns, lb + 60, [[3, 2]]),
                _sv(bns, lb + 61, [[3, 2]]),
                _sv(bns, lb + 62, [[3, 2]]),
                le + 20, 2,
            )
            t9 = main.tile([P, 2], F32)
            nc.vector.tensor_reduce(
                out=t9[:, 0:1], in_=cm[:, le + 20 : le + 22],
                axis=mybir.AxisListType.X, op=ALU.add,
            )
            nc.vector.tensor_reduce(
                out=t9[:, 1:2], in_=s2[:, le + 20 : le + 22],
                axis=mybir.AxisListType.X, op=ALU.add,
            )
            nc.vector.tensor_tensor(out=sums[:, 3:4], in0=psum3, in1=t9[:, 0:1], op=ALU.add)
            nc.vector.tensor_tensor(out=ssqs[:, 3:4], in0=pssq3, in1=t9[:, 1:2], op=ALU.add)

            # ---- scalar tail, run per column-batch (0:3 early, 3:4 late)
            t0_ = main.tile([P, T], F32)
            v1 = main.tile([P, T], F32)
            std = main.tile([P, T], F32)
            den = main.tile([P, T], F32)
            b1 = main.tile([P, T], F32)
            rec = main.tile([P, T], F32)
            b2 = main.tile([P, T], F32)
            d1 = main.tile([P, T], F32)
            aab = main.tile([P, T], F32)
            b3 = main.tile([P, T], F32)
            m1 = main.tile([P, T], mybir.dt.uint8)
            m2 = main.tile([P, T], mybir.dt.uint8)
            r1 = main.tile([P, T], F32)
            realism = main.tile([P, T], F32)
            den2 = main.tile([P, T], F32)
            rden = main.tile([P, T], F32)
            bnd = main.tile([P, T], F32)
            final = main.tile([P, T], F32)

            def chain(a, b):
                sl = slice(a, b)
                # var*(D-1) = sumsq - sum^2/D ; std = sqrt(v1 / (D-1))
                nc.vector.scalar_tensor_tensor(
                    out=t0_[:, sl], in0=sums[:, sl], scalar=1.0 / D,
                    in1=sums[:, sl], op0=ALU.mult, op1=ALU.mult,
                )
                nc.vector.tensor_tensor(
                    out=v1[:, sl], in0=ssqs[:, sl], in1=t0_[:, sl], op=ALU.subtract
                )
                nc.scalar.activation(
                    out=std[:, sl], in_=v1[:, sl], func=ACTF.Sqrt,
                    scale=1.0 / (D - 1),
                )
                # den = sqrt(startsq * |ctx|^2) on ACT (scale = per-part cn2)
                nc.scalar.activation(
                    out=den[:, sl], in_=startsq[:, sl], func=ACTF.Sqrt, scale=cn2
                )
                # piecewise realism (+0.15 regime term folded in)
                nc.vector.tensor_scalar(
                    out=b1[:, sl], in0=std[:, sl], scalar1=10.0, scalar2=0.15,
                    op0=ALU.mult, op1=ALU.add,
                )
                nc.vector.reciprocal(out=rec[:, sl], in_=std[:, sl])
                nc.vector.tensor_scalar(
                    out=b2[:, sl], in0=rec[:, sl], scalar1=0.5, scalar2=0.15,
                    op0=ALU.mult, op1=ALU.add,
                )
                nc.vector.tensor_scalar(
                    out=d1[:, sl], in0=std[:, sl], scalar1=0.1, scalar2=None,
                    op0=ALU.subtract,
                )
                nc.vector.scalar_tensor_tensor(
                    out=aab[:, sl], in0=d1[:, sl], scalar=-1.0, in1=d1[:, sl],
                    op0=ALU.mult, op1=ALU.max,
                )
                nc.vector.tensor_scalar(
                    out=b3[:, sl], in0=aab[:, sl], scalar1=-1.0, scalar2=1.15,
                    op0=ALU.mult, op1=ALU.add,
                )
                nc.vector.tensor_scalar(
                    out=m1[:, sl], in0=std[:, sl], scalar1=0.01, scalar2=None,
                    op0=ALU.is_lt,
                )
                nc.vector.tensor_scalar(
                    out=m2[:, sl], in0=std[:, sl], scalar1=0.5, scalar2=None,
                    op0=ALU.is_gt,
                )
                nc.vector.select(
                    out=r1[:, sl], mask=m2[:, sl], on_true=b2[:, sl],
                    on_false=b3[:, sl],
                )
                nc.vector.select(
                    out=realism[:, sl], mask=m1[:, sl], on_true=b1[:, sl],
                    on_false=r1[:, sl],
                )
                # boundary = num / max(den, eps)
                nc.vector.tensor_scalar(
                    out=den2[:, sl], in0=den[:, sl], scalar1=EPS, scalar2=None,
                    op0=ALU.max,
                )
                nc.vector.reciprocal(out=rden[:, sl], in_=den2[:, sl])
                nc.vector.tensor_tensor(
                    out=bnd[:, sl], in0=nums[:, sl], in1=rden[:, sl], op=ALU.mult
                )
                nc.vector.scalar_tensor_tensor(
                    out=final[:, sl], in0=bnd[:, sl], scalar=0.2,
                    in1=realism[:, sl], op0=ALU.mult, op1=ALU.add,
                )

            with w03 if CHAIN03_WAIT_MS <= 0 else tc.tile_wait_until(CHAIN03_WAIT_MS):
                chain(0, 3)
                nc.sync.dma_start(out=out[:, 0:3], in_=final[:, 0:3])
            chain(3, 4)
            nc.sync.dma_start(out=out[:, 3:4], in_=final[:, 3:4])
    nc.compile()
    return nc


_NC_CACHE = None


def _get_nc() -> bass.Bass:
    global _NC_CACHE
    if _NC_CACHE is None:
        _NC_CACHE = _build()
    return _NC_CACHE


def run(inputs: dict, trace: bool = False, **kw):
    """Returns (output [4096] f32, BassKernelResults)."""
    chunks = np.ascontiguousarray(np.asarray(inputs["chunks"], dtype=np.float32))
    pc = np.asarray(inputs["previous_context"], dtype=np.float32)
    ctx = np.ascontiguousarray(pc[-10:].reshape(-1))
    assert chunks.shape == (N_TOTAL, 128, 64)
    assert ctx.shape == (S,)

    nc = _get_nc()
    in_maps = [
        {"chunks": chunks[c * N_LOC : (c + 1) * N_LOC], "ctx": ctx}
        for c in range(N_CORES)
    ]
    res = run_bass_kernel_spmd(nc, in_maps, core_ids=list(range(N_CORES)),
                               trace=trace, **kw)
    # out[p, t] = score of local chunk t*128+p -> transpose to chunk order
    full = np.concatenate([r["out"].T.reshape(-1) for r in res.results])
    return full.astype(np.float32), res


def kernel(**inputs) -> np.ndarray:
    return run(inputs)[0]
